# revision 10
# baseline (speedup 1.0000x reference)
"""GATv2Conv batched-graph kernel for Trainium2 (8 NeuronCores, data-parallel).

Problem: B=16384 independent 9-node graphs, C_in=C_out=256, fixed edge list
(16 directed tree edges + 9 self-loops = 25 edges), GATv2 attention.

Per core (B/8 = 2048 graphs), per block of G=512 graphs:
  - host pre-transposes x to channel-major node-major layout
    xT[c, blk*9*G + s*G + g] (bf16) so all matmuls/vector slices are
    contiguous.
  - PE: channel-major xlT/xrT projections (lhsT = W chunks); graph-major
    xl_gm projection (lhsT = xT slices, rhs = W) for the aggregation;
    nu = [att@W_l | att@W_r] . x per node (tiny one-hot-block stationaries);
    per-edge score matmuls att*relu(s_e) accumulated into one [25, G] PSUM
    tile via att (x) onehot(e) stationaries + one selector matmul adding the
    0.2*att*s_e linear part (leaky(x) = 0.2x + 0.8 relu(x), with 0.8/0.2
    folded into the constants); denominator segment-sum via a [25, 9]
    indicator matmul; small transposes for per-graph alpha.
  - DVE/GPSIMD: per-edge adds (bf16 2x), relu (tensor_scalar 4x), and the
    alpha-weighted aggregation with fused scalar_tensor_tensor ops in
    graph-major layout (alpha is a per-partition scalar there).
  - ACT: exp + most PSUM->SBUF copies.
  - softmax needs no max-subtraction (scores are O(1)); bias is handled
    host-side (it is zeros in this problem).
"""

import sys

if "/opt/trn_rl_repo" not in sys.path:
    sys.path.insert(0, "/opt/trn_rl_repo")

import numpy as np
import ml_dtypes

import concourse.bass as bass
import concourse.bacc as bacc
import concourse.mybir as mybir
from concourse import tile
from concourse.bass_utils import run_bass_kernel_spmd

F32 = mybir.dt.float32
BF16 = mybir.dt.bfloat16

N_CORES = 8
B_TOTAL = 16384
NEG_SLOPE = 0.2
BC = B_TOTAL // N_CORES          # graphs per core
NN = 9                           # nodes per graph
C = 256                          # channels
G = 512                          # graphs per block
NBLK = BC // G                   # blocks per core
NT = G // 128                    # 128-graph subtiles per block
NGT = NN * G                     # columns per (chunk, block)

# ---- static edge list, grouped by destination, self-loop first in group ----
_ADJ = {0: [1, 3, 5, 7], 1: [0, 2], 2: [1], 3: [0, 4], 4: [3],
        5: [0, 6], 6: [5], 7: [0, 8], 8: [7]}
EDGES = []          # (src, dst), sorted by dst, self-loop first
GRP_OFF = [0]       # group offsets into EDGES per dst
for _d in range(NN):
    EDGES.append((_d, _d))
    for _s in _ADJ[_d]:
        EDGES.append((_s, _d))
    GRP_OFF.append(len(EDGES))
NE = len(EDGES)     # 25
assert NE == 25


class Cfg:
    add_engines = ("vector", "gpsimd")       # per-edge adds round robin
    relu_engines = ("vector", "scalar")      # per-edge relu round robin
    agg_engines = ("vector",)                # agg STT round robin (AP scalar
                                             # ops are DVE-only)
    copy_engines = ("scalar",)               # psum->sbuf copies (ACT gap-fills)
    aggi_engines = ("scalar", "scalar", "vector")  # agg init: copy with scale
    proj_pair = 2                            # 512-col groups per proj psum


def build_program(cfg: Cfg):
    nc = bacc.Bacc("TRN2", target_bir_lowering=False, debug=False)

    def eng(name):
        return {"vector": nc.vector, "gpsimd": nc.gpsimd,
                "scalar": nc.scalar}[name]

    def copy_op(ename, dst_ap, src_ap):
        if ename == "scalar":
            nc.scalar.copy(dst_ap, src_ap)
        else:
            eng(ename).tensor_copy(dst_ap, src_ap)

    # DRAM tensors
    xT_d = nc.dram_tensor("xT", [C, NBLK * NGT], BF16, kind="ExternalInput")
    wl_d = nc.dram_tensor("wl", [C, C], BF16, kind="ExternalInput")
    wr_d = nc.dram_tensor("wr", [C, C], BF16, kind="ExternalInput")
    attbl_d = nc.dram_tensor("attbl", [128, 2 * NE * NE], BF16,
                             kind="ExternalInput")
    smat_d = nc.dram_tensor("smat", [NE, NE], F32, kind="ExternalInput")
    identf_d = nc.dram_tensor("identf", [128, 128], F32, kind="ExternalInput")
    out_d = nc.dram_tensor("out", [BC, NN * C], BF16, kind="ExternalOutput")

    with tile.TileContext(nc) as tc:
        with (
            tc.tile_pool(name="const", bufs=1) as cpool,
            tc.tile_pool(name="xin", bufs=2) as xpool,
            tc.tile_pool(name="proj", bufs=2) as prpool,
            tc.tile_pool(name="edge", bufs=10) as epool,
            tc.tile_pool(name="soft", bufs=3) as spool,
            tc.tile_pool(name="gm", bufs=6) as gmpool,
            tc.tile_pool(name="outp", bufs=4) as opool,
            tc.tile_pool(name="ps_proj", bufs=2, space="PSUM") as ps_proj,
            tc.tile_pool(name="ps_gm", bufs=2, space="PSUM") as ps_gm,
            tc.tile_pool(name="ps_sc", bufs=1, space="PSUM") as ps_sc,
            tc.tile_pool(name="ps_misc", bufs=1, space="PSUM") as ps_misc,
        ):
            # ---- constants ----
            wl_sb = cpool.tile([128, 2 * C], BF16, tag="wl")
            wr_sb = cpool.tile([128, 2 * C], BF16, tag="wr")
            nc.sync.dma_start(wl_sb[:, 0:C], wl_d[0:128, :])
            nc.sync.dma_start(wl_sb[:, C:2 * C], wl_d[128:256, :])
            nc.sync.dma_start(wr_sb[:, 0:C], wr_d[0:128, :])
            nc.sync.dma_start(wr_sb[:, C:2 * C], wr_d[128:256, :])
            attbl_sb = cpool.tile([128, 2 * NE * NE], BF16, tag="attbl")
            nc.sync.dma_start(attbl_sb[:], attbl_d[:])
            smat_sb = cpool.tile([NE, NE], F32, tag="smat")
            nc.sync.dma_start(smat_sb[:], smat_d[:])
            identf_sb = cpool.tile([128, 128], F32, tag="identf")
            nc.sync.dma_start(identf_sb[:], identf_d[:])
            zero_sb = cpool.tile([128, G], BF16, tag="zero")
            nc.gpsimd.memset(zero_sb[:], 0.0)

            ci = 0   # copy engine cycler
            for b in range(NBLK):
                # ---- load xT block: 2 channel-chunks [128, NN*G] ----
                xt = []
                for cc in range(2):
                    t = xpool.tile([128, NGT], BF16, tag=f"xt{cc}")
                    nc.sync.dma_start(
                        t[:], xT_d[cc * 128:(cc + 1) * 128,
                                   b * NGT:(b + 1) * NGT])
                    xt.append(t)

                # ---- channel-major projections, one tile per psum copy ----
                # fine-grained tiles so edge adds gate on single copies
                xlT = {}   # (dch, s) -> (tile, col offset)
                xrT = {}
                for (wsb, dest, nm) in ((wl_sb, xlT, "l"), (wr_sb, xrT, "r")):
                    for dch in range(2):
                        s = 0
                        while s < NN:
                            npair = min(cfg.proj_pair, NN - s)
                            ps = ps_proj.tile([128, cfg.proj_pair * G], F32,
                                              tag="ps_proj")
                            for j in range(npair):
                                nc.tensor.matmul(
                                    ps[:, j * G:(j + 1) * G],
                                    wsb[:, dch * 128:dch * 128 + 128],
                                    xt[0][:, (s + j) * G:(s + j + 1) * G],
                                    start=True, stop=False)
                                nc.tensor.matmul(
                                    ps[:, j * G:(j + 1) * G],
                                    wsb[:, C + dch * 128:C + dch * 128 + 128],
                                    xt[1][:, (s + j) * G:(s + j + 1) * G],
                                    start=False, stop=True)
                            dst = prpool.tile(
                                [128, npair * G], BF16,
                                tag=f"p{nm}{dch}{s}")
                            for j in range(npair):
                                dest[(dch, s + j)] = (dst, j * G)
                            copy_op(cfg.copy_engines[ci % len(cfg.copy_engines)],
                                    dst[:], ps[:, 0:npair * G])
                            ci += 1
                            s += npair

                # ---- graph-major xl for aggregation (stationary = xT) ----
                # emitted as closures interleaved into the edge loop so PE
                # has ready work while score matmuls wait on DVE-made inputs
                xl_gms = [gmpool.tile([128, NN * C], BF16, tag="xl_gm",
                                      name="xl_gm")
                          for _ in range(NT)]

                def gm_piece(t, s, ce):
                    npair = min(2, NN - s)
                    ps = ps_gm.tile([128, 512], F32, tag="ps_gm")
                    for j in range(npair):
                        nc.tensor.matmul(
                            ps[:, j * C:(j + 1) * C],
                            xt[0][:, (s + j) * G + t * 128:
                                  (s + j) * G + (t + 1) * 128],
                            wl_sb[:, 0:C],
                            start=True, stop=False)
                        nc.tensor.matmul(
                            ps[:, j * C:(j + 1) * C],
                            xt[1][:, (s + j) * G + t * 128:
                                  (s + j) * G + (t + 1) * 128],
                            wl_sb[:, C:2 * C],
                            start=False, stop=True)
                    copy_op(ce, xl_gms[t][:, s * C:(s + npair) * C],
                            ps[:, 0:npair * C])

                gm_pieces = [(t, s) for t in range(NT)
                             for s in (0, 2, 4, 6, 8)]

                # ---- edge phase: fused leaky(xl+xr) + score matmuls ----
                sc_ps = ps_sc.tile([NE, G], F32, tag="ps_sc")
                for e, (s, d) in enumerate(EDGES):
                    if e < len(gm_pieces):
                        gt, gs = gm_pieces[e]
                        gm_piece(gt, gs,
                                 cfg.copy_engines[ci % len(cfg.copy_engines)])
                        ci += 1
                    for dch in range(2):
                        idx = e * 2 + dch
                        st = epool.tile([128, G], BF16, tag="st")
                        ae = cfg.add_engines[idx % len(cfg.add_engines)]
                        lt, lo = xlT[(dch, s)]
                        rt, ro = xrT[(dch, d)]
                        eng(ae).tensor_tensor(
                            st[:], lt[:, lo:lo + G], rt[:, ro:ro + G],
                            op=mybir.AluOpType.add)
                        lk = epool.tile([128, G], BF16, tag="lk")
                        re = cfg.relu_engines[idx % len(cfg.relu_engines)]
                        if re == "scalar":
                            nc.scalar.activation(
                                lk[:], st[:],
                                mybir.ActivationFunctionType.Prelu,
                                alpha=NEG_SLOPE)
                        else:
                            # leaky(x) = max(0.2*x, x)
                            eng(re).scalar_tensor_tensor(
                                lk[:], st[:], NEG_SLOPE, st[:],
                                op0=mybir.AluOpType.mult,
                                op1=mybir.AluOpType.max)
                        blk = (dch * NE + e) * NE
                        nc.tensor.matmul(
                            sc_ps[:], attbl_sb[:, blk:blk + NE], lk[:],
                            start=(e == 0 and dch == 0),
                            stop=(e == NE - 1 and dch == 1))

                # ---- softmax (no max-subtraction; scores are O(1)) ----
                ex_sb = spool.tile([NE, G], F32, tag="ex")
                nc.scalar.activation(ex_sb[:], sc_ps[:],
                                     mybir.ActivationFunctionType.Exp)
                # per-edge gathered denominator: M[e',e] = [dst match]
                den_ps = ps_misc.tile([NE, G], F32, tag="ps_misc")
                nc.tensor.matmul(den_ps[:], smat_sb[:], ex_sb[:],
                                 start=True, stop=True)
                den_sb = spool.tile([NE, G], F32, tag="den")
                nc.scalar.copy(den_sb[:], den_ps[:])

                # transposed per-128-graph alpha: one TT per block
                exT_ps = ps_misc.tile([128, 2 * NT * NE], F32,
                                      tag="ps_misc")
                dT0 = NT * NE
                for t in range(NT):
                    nc.tensor.transpose(
                        exT_ps[:, t * NE:(t + 1) * NE],
                        ex_sb[:, t * 128:(t + 1) * 128],
                        identf_sb[0:NE, 0:NE])
                    nc.tensor.transpose(
                        exT_ps[:, dT0 + t * NE:dT0 + (t + 1) * NE],
                        den_sb[:, t * 128:(t + 1) * 128],
                        identf_sb[0:NE, 0:NE])
                rdenT = spool.tile([128, NT * NE], F32, tag="rdenT")
                nc.vector.reciprocal(rdenT[:], exT_ps[:, dT0:dT0 + 2 * dT0 - dT0])
                alphaT = spool.tile([128, NT * NE], F32, tag="alphaT")
                nc.vector.tensor_tensor(
                    alphaT[:], exT_ps[:, 0:dT0], rdenT[:],
                    op=mybir.AluOpType.mult)

                # ---- aggregation in graph-major ----
                gi = 0
                for t in range(NT):
                    xl_gm = xl_gms[t]
                    out_t = opool.tile([128, NN * C], BF16, tag="out_t")
                    for d in range(NN):
                        o0 = GRP_OFF[d]
                        ie = cfg.aggi_engines[gi % len(cfg.aggi_engines)]
                        if ie == "scalar":
                            nc.scalar.activation(
                                out_t[:, d * C:(d + 1) * C],
                                xl_gm[:, d * C:(d + 1) * C],
                                mybir.ActivationFunctionType.Copy,
                                scale=alphaT[:, t * NE + o0:t * NE + o0 + 1])
                        else:
                            eng(ie).tensor_scalar_mul(
                                out_t[:, d * C:(d + 1) * C],
                                xl_gm[:, d * C:(d + 1) * C],
                                alphaT[:, t * NE + o0:t * NE + o0 + 1])
                        for e in range(o0 + 1, GRP_OFF[d + 1]):
                            s = EDGES[e][0]
                            ge = cfg.agg_engines[gi % len(cfg.agg_engines)]
                            gi += 1
                            eng(ge).scalar_tensor_tensor(
                                out_t[:, d * C:(d + 1) * C],
                                xl_gm[:, s * C:(s + 1) * C],
                                alphaT[:, t * NE + e:t * NE + e + 1],
                                out_t[:, d * C:(d + 1) * C],
                                op0=mybir.AluOpType.mult,
                                op1=mybir.AluOpType.add)
                    nc.sync.dma_start(
                        out_d[b * G + t * 128:b * G + (t + 1) * 128, :],
                        out_t[:])

    nc.compile()
    return nc


def make_host_inputs(x, W_l, W_r, att, cfg: Cfg):
    """Builds the per-core input maps (host-side sharding + layout prep)."""
    x = np.asarray(x, dtype=np.float32)
    W_l = np.ascontiguousarray(np.asarray(W_l, dtype=np.float32))
    W_r = np.ascontiguousarray(np.asarray(W_r, dtype=np.float32))
    att = np.asarray(att, dtype=np.float32)
    bf = ml_dtypes.bfloat16

    # att (x) onehot(e) stationary blocks for the leaky-relu'd score dot
    attbl = np.zeros((128, 2, NE, NE), dtype=np.float32)
    for dch in range(2):
        for e in range(NE):
            attbl[:, dch, e, e] = att[dch * 128:(dch + 1) * 128]
    attbl = attbl.reshape(128, 2 * NE * NE).astype(bf)

    smat = np.zeros((NE, NE), dtype=np.float32)
    for e1, (_s1, d1) in enumerate(EDGES):
        for e2, (_s2, d2) in enumerate(EDGES):
            if d1 == d2:
                smat[e1, e2] = 1.0

    ident = np.eye(128, dtype=np.float32)

    in_maps = []
    for c in range(N_CORES):
        xc = x[c * BC:(c + 1) * BC]                       # [BC, 9, 256]
        xT = np.ascontiguousarray(
            xc.reshape(NBLK, G, NN, C).transpose(3, 0, 2, 1).reshape(
                C, NBLK * NGT).astype(bf))
        in_maps.append({
            "xT": xT,
            "wl": W_l.astype(bf),
            "wr": W_r.astype(bf),
            "attbl": attbl,
            "smat": smat,
            "identf": ident,
        })
    return in_maps


_CACHE = {}


def _cfg_key(cfg: Cfg):
    return (cfg.add_engines, cfg.relu_engines, cfg.agg_engines,
            cfg.copy_engines, cfg.aggi_engines, cfg.proj_pair)


def _get_program(cfg: Cfg):
    key = _cfg_key(cfg)
    if key not in _CACHE:
        _CACHE[key] = build_program(cfg)
    return _CACHE[key]


def kernel(x, W_l, W_r, att, bias, cfg: Cfg = None, trace: bool = False,
           _results_holder: dict = None, **run_kwargs):
    cfg = cfg or Cfg()
    nc = _get_program(cfg)
    in_maps = make_host_inputs(x, W_l, W_r, att, cfg)
    res = run_bass_kernel_spmd(nc, in_maps, core_ids=list(range(N_CORES)),
                               trace=trace, **run_kwargs)
    if _results_holder is not None:
        _results_holder["res"] = res
    outs = [np.asarray(r["out"], dtype=np.float32).reshape(BC, NN, C)
            for r in res.results]
    out = np.concatenate(outs, axis=0)
    bias = np.asarray(bias, dtype=np.float32)
    if np.any(bias):
        out = out + bias
    return out.astype(np.float32)

